# revision 21
# baseline (speedup 1.0000x reference)
"""Trainium2 bass kernel for nn_CM_41162966565199 (dense_cnn, dynamic filter).

Computation (per batch sample):
  filt = Conv2d(C=64 -> 9C=576, 3x3, pad=1)(gt) + bias          # dynamic filters
  out[c,h,w] = sum_j filt[c*9+j, h, w] * patches_j(gr)[c, h, w] # 3x3 dyn. filter

Strategy: pure data parallel, one sample per NeuronCore (N=8, 8 cores).

Per core:
- Conv as shift-based bf16 matmuls (1 cycle/row, same PE rate as fp32r here
  but half the window DMA bytes): contraction (in_channel i, tap p) tiled
  into 5 K=128 chunks by pairing taps whose flat-offset delta is +1 (or
  +130), realized by stacking two shifted copies of gt on SBUF partitions
  0-63 / 64-127. Output channels (c, j) tiled into 5 M-tiles of two
  j-groups each. All matmuls K=128, M=128, N=512.
- Compact 1-ring 130x130 grid: every 3x3 tap is a pure flat offset, and the
  output is addressed as out[h*128+w] via 3D row-strided APs
  ([4 rows @ 130][128 @ 1] per 512-wide tile), so the 32 output tiles map
  exactly onto the 128x128 image -- no padded-row compute waste and no
  host-side crop (just a reshape).
- Dynamic-filter stage: DVE scalar_tensor_tensor fuses (psum + bias) * gr
  reading PSUM directly; the pairwise fp16 add tree is split DVE/GpSimd
  (Pool takes 2 of the 4.5 adds). Upper/lower partition halves hold
  disjoint partial sums, folded on the host.
- A dozen-odd warm-up matmuls on the weight tile run during the head DMA
  so the PE p-state ramp (full clock only after 3us of continuous busy)
  completes before the first real matmul.
"""

import numpy as np
import ml_dtypes

import concourse.bass as bass
import concourse.mybir as mybir
import concourse.tile as tile
from concourse import bacc
from concourse.bass import AP
from concourse.bass_utils import run_bass_kernel_spmd
from concourse.vector_clock import ScopedClock

# ------------------------------------------- deterministic walrus codegen
# walrus --jobs 8 codegen is nondeterministic and lands ~half of compiles in
# a NEFF whose PE instruction stream runs ~20% slower (239-260ns vs 216ns
# per 512-row matmul). Serial codegen deterministically produces the fast
# stream.
import concourse.bass_utils as _bu

_orig_run_command = _bu.run_command


def _serial_walrus_run_command(argv, **kwargs):
    argv = list(argv)
    for _i, _a in enumerate(argv):
        if _a == "--jobs" and _i + 1 < len(argv):
            argv[_i + 1] = "1"
    return _orig_run_command(argv, **kwargs)


_bu.run_command = _serial_walrus_run_command

# ---------------------------------------------------------------- constants
N, C, H, W, KS = 8, 64, 128, 128, 3
G = W + 2                       # 130: 1-ring padded row width
GRID = G * G                    # 16900 flat grid
NTILE = 512                     # output cols per tile = 4 image rows
NT = (H * W) // NTILE           # 32 output tiles, no spatial waste
OUT_LEN = NT * NTILE            # 16384
FLAT_SRC = 18944                # padded flat source length (covers max reads)
ROWB = 4 * G                    # 520: grid cols consumed per output tile

BF16 = ml_dtypes.bfloat16

F32 = mybir.dt.float32
F16 = mybir.dt.float16
BF = mybir.dt.bfloat16
ADD = mybir.AluOpType.add
MULT = mybir.AluOpType.mult

# 5 K-chunks over the 9 conv taps p=(kh,kw); flat offset d_p = kh*130+kw.
# Pairs (p_a, p_b) stacked on lower/upper SBUF partition halves. Chunks 0-2
# pair (kh,0)+(kh,1) (delta=1, "ab" region), chunk 3 pairs (0,2)+(1,2)
# (delta=130, "ac" region), chunk 4 is the lone (2,2) with zeroed upper
# weights reading the ab region at +262.
CHUNKS = [((0, 0), (0, 1)), ((1, 0), (1, 1)), ((2, 0), (2, 1)),
          ((0, 2), (1, 2)), ((2, 2), None)]
MTILES = CHUNKS  # M-tiles: which two j-groups share a PSUM tile's halves
# per-chunk rhs source: (region, extra offset); region 0=ab(s|s+1),
# 1=ac(s+2|s+132)
CHUNK_SRC = [(0, 0), (0, 130), (0, 260), (1, 0), (0, 262)]
CHUNK_ORDER = [0, 1, 2, 4, 3]   # ac-chunk last: its window DMA lands late
MT_SRC = CHUNK_SRC              # gr operand mapping per M-tile (same pairs)


# ------------------------------------------------- TileContext drain patch
# This walrus build rejects >2 sync-wait commands on one CTRL instruction;
# the stock TileContext tail hangs every pending sem wait on a single SP
# Drain. Split them across single-wait SP NOPs (program order on SP still
# places them before the barrier + sem reset).
def _drain_and_barrier_split(self, tick_clock, wait_clock):
    nc = self.nc
    drain_inst = nc.sync.drain()
    wait_clock.add_sem_waits(
        drain_inst.ins, ScopedClock({None: tick_clock.global_clock})
    )
    si = drain_inst.ins.sync_info
    if si is not None and len(si.on_wait) > 1:
        waits = list(si.on_wait)
        drain_inst.ins.sync_info = mybir.SyncInfo(on_wait=[waits[0]], on_update=[])
        for w in waits[1:]:
            nop = nc.sync.nop()
            nop.ins.sync_info = mybir.SyncInfo(on_wait=[w], on_update=[])
    nc.all_engine_barrier()
    assert self.sems is not None
    popped = nc._tile_sem_poison_stack.pop()
    assert popped is self._sem_poison
    nc.clear_and_free_semaphores(list(self.sems.allocated().values()))
    nc.all_engine_barrier()


tile.TileContext._drain_and_barrier = _drain_and_barrier_split


# ------------------------------------------------------------- host prep
def _prep_gt(gt):
    """[C,H,W] -> [C, FLAT_SRC] bf16 flat 130x130 grid, 1-ring zero pad."""
    buf = np.zeros((C, FLAT_SRC), np.float32)
    pad = np.zeros((C, G, G), np.float32)
    pad[:, 1:1 + H, 1:1 + W] = gt
    buf[:, :GRID] = pad.reshape(C, -1)
    return buf.astype(BF16)


def _prep_gr(gr):
    """[C,H,W] -> [C, FLAT_SRC] fp16 flat grid; 1-ring replicate pad."""
    rp = np.pad(gr, ((0, 0), (1, 1), (1, 1)), mode="edge")
    buf = np.zeros((C, FLAT_SRC), np.float32)
    buf[:, :GRID] = rp.reshape(C, -1)
    return buf.astype(np.float16)


def _jidx(j):
    return j[0] * 3 + j[1]


def _prep_w(Wc):
    """[576,64,3,3] -> [128, 25*128] bf16 lhsT blocks [(m,chunk), K, M]."""
    out = np.zeros((5, 5, 128, 128), np.float32)
    cc = np.arange(C)
    for m, (j0, j1) in enumerate(MTILES):
        for c, (pa, pb) in enumerate(CHUNKS):
            for hk, p in ((0, pa), (1, pb)):
                if p is None:
                    continue
                kh, kw = p
                for hm, j in ((0, j0), (1, j1)):
                    if j is None:
                        continue
                    blk = Wc[cc * 9 + _jidx(j), :, kh, kw]  # [c_out, i]
                    out[m, c, 64 * hk:64 * hk + 64, 64 * hm:64 * hm + 64] = blk.T
    # partition-major [128, 25*128] so the device load is plain 2D DMAs
    return np.ascontiguousarray(
        out.reshape(25, 128, 128).transpose(1, 0, 2).reshape(128, 25 * 128)
    ).astype(BF16)


def _prep_b(bc):
    """[576] -> [128,5] per-M-tile per-partition bias (partition-major)."""
    out = np.zeros((5, 128), np.float32)
    cc = np.arange(C)
    for m, (j0, j1) in enumerate(MTILES):
        for hm, j in ((0, j0), (1, j1)):
            if j is None:
                continue
            out[m, 64 * hm:64 * hm + 64] = bc[cc * 9 + _jidx(j)]
    return np.ascontiguousarray(out.T)


# --------------------------------------------------------- bass program
def _build():
    # Bacc (not plain Bass): its finalize() -> compile() legalizes the
    # multi-wait instructions Tile emits (move_matmul_waits_to_ldweights,
    # generate_event_semaphores) which this walrus build otherwise rejects
    # with "Too many sync wait commands".
    nc = bacc.Bacc(None, target_bir_lowering=False)
    gt_src = nc.dram_tensor("gt_src", [C, FLAT_SRC], BF, kind="ExternalInput")
    gr_src = nc.dram_tensor("gr_src", [C, FLAT_SRC], F16, kind="ExternalInput")
    w_src = nc.dram_tensor("w_src", [128, 25 * 128], BF, kind="ExternalInput")
    b_src = nc.dram_tensor("b_src", [128, 5], F32, kind="ExternalInput")
    o_dst = nc.dram_tensor("o_dst", [128, OUT_LEN], F16, kind="ExternalOutput")

    # graduated block sizes: a tiny first block gets the PE computing real
    # tiles ~9us earlier; later blocks amortize window overlap. Each block's
    # windows prefetch during the previous block's compute.
    sizes = [2, 2, 6, 8, 8, 6]
    assert sum(sizes) == NT
    blocks = []
    t0 = 0
    for nb in sizes:
        blocks.append((t0, nb))
        t0 += nb
    WREG = max(nb * ROWB + 260 for nb in sizes)  # 6500

    with tile.TileContext(nc) as tc:
        with (
            tc.tile_pool(name="wpool", bufs=1) as wpool,
            tc.tile_pool(name="winpool", bufs=2) as winpool,
            tc.tile_pool(name="pspool", bufs=4, space="PSUM") as pspool,
            tc.tile_pool(name="prodpool", bufs=12) as prodpool,
            tc.tile_pool(name="prod4pool", bufs=4) as prod4pool,
            tc.tile_pool(name="accpool", bufs=6) as accpool,
        ):
            wsb = wpool.tile([128, 25 * 128], BF, name="wsb", tag="wsb")
            bias_sb = wpool.tile([128, 5], F32, name="bias_sb", tag="bias")
            warm_sb = wpool.tile([128, NTILE], BF, name="warm_sb", tag="warm")

            def load_weights_m(m):
                nc.sync.dma_start(
                    out=wsb[:, m * 640:(m + 1) * 640],
                    in_=w_src[:, m * 640:(m + 1) * 640],
                )

            # PE p-state warm-up on a memset tile (no DMA dependency, so it
            # starts right after the framework preamble; full clock needs
            # 3us of continuous PE busy). Results recycle, never read.
            nc.gpsimd.memset(warm_sb[:, :], 0.0)
            warm_ps = pspool.tile([128, 2 * NTILE], F32, name="warm", tag="ps")
            for _ in range(12):
                nc.tensor.matmul(
                    warm_ps[:, 0:NTILE], warm_sb[:, 0:128], warm_sb[:, :],
                    start=True, stop=True)

            for bi, (t0, nb) in enumerate(blocks):
                T = t0 * ROWB
                wneed = nb * ROWB + 260
                gtw = winpool.tile([128, 2, WREG], BF, name="gtw", tag="gtw")
                grw = winpool.tile([128, 2, WREG], F16, name="grw", tag="grw")
                for r, offs in enumerate(((0, 1), (2, 132))):
                    for h, off in enumerate(offs):
                        nc.sync.dma_start(
                            out=gtw[64 * h:64 * h + 64, r, 0:wneed],
                            in_=gt_src[:, T + off:T + off + wneed])
                    if bi == 0 and r == 0:
                        # m0's weights right behind the ab halves: the DMA
                        # pool drains FIFO, so the first real matmul's inputs
                        # land before the rest of the head traffic
                        load_weights_m(0)
                if bi == 0:
                    # remaining weights early: they must precede the next
                    # block's big window loads or m1+ matmuls stall on
                    # weight sems
                    for m in range(1, 5):
                        load_weights_m(m)
                    nc.sync.dma_start(out=bias_sb[:, :], in_=b_src[:, :])
                for r, offs in enumerate(((0, 1), (2, 132))):
                    for h, off in enumerate(offs):
                        nc.sync.dma_start(
                            out=grw[64 * h:64 * h + 64, r, 0:wneed],
                            in_=gr_src[:, T + off:T + off + wneed])

                gtw_t = gtw[:, :, :].tensor
                grw_t = grw[:, :, :].tensor
                npair = nb // 2
                Wd = 2 * NTILE
                prods = [[None] * 5 for _ in range(npair)]
                accs = [[None, None] for _ in range(npair)]
                for m in range(5):
                    pst = [
                        pspool.tile([128, Wd], F32, name=f"ps{m}_{p}",
                                    tag="ps")
                        for p in range(npair)
                    ]
                    for p in range(npair):
                        for tb in range(2):
                            y = (2 * p + tb) * ROWB
                            for ci, c in enumerate(CHUNK_ORDER):
                                reg, xtra = CHUNK_SRC[c]
                                k = m * 5 + c
                                rhs = AP(gtw_t, reg * WREG + y + xtra,
                                         [[2 * WREG, 128], [G, 4], [1, W]])
                                nc.tensor.matmul(
                                    pst[p][:, tb * NTILE:(tb + 1) * NTILE],
                                    wsb[:, k * 128:(k + 1) * 128], rhs,
                                    start=(ci == 0), stop=(ci == 4),
                                )
                    # drain PSUM: stt fuses (psum + bias) * gr on DVE
                    for p in range(npair):
                        reg, moff = MT_SRC[m]
                        gbase = reg * WREG + p * 2 * ROWB + moff
                        if m < 4:
                            pr = prodpool.tile([128, Wd], F16, name=f"m{m}",
                                               tag="prod")
                            gap = AP(grw_t, gbase,
                                     [[2 * WREG, 128], [G, 8], [1, W]])
                            nc.vector.scalar_tensor_tensor(
                                pr[:, :], pst[p][:, :], bias_sb[:, m:m + 1],
                                gap, op0=ADD, op1=MULT)
                        else:
                            pr = prod4pool.tile([64, Wd], F16, name="m4",
                                                tag="prod4")
                            gap = AP(grw_t, gbase,
                                     [[2 * WREG, 64], [G, 8], [1, W]])
                            nc.vector.scalar_tensor_tensor(
                                pr[:, :], pst[p][0:64, :],
                                bias_sb[0:64, 4:5], gap, op0=ADD, op1=MULT)
                        prods[p][m] = pr

                    # fp16 pairwise add tree on DVE, emitted inside the
                    # m-loop so prod tiles release promptly (prevents
                    # prod-ring <-> PSUM-recycle deadlock) and so only the
                    # m4-dependent final add trails the last matmul
                    if m == 1:
                        for p in range(npair):
                            a1 = accpool.tile([128, Wd], F16, name="a1",
                                              tag="acc")
                            nc.vector.tensor_tensor(
                                a1[:, :], prods[p][0][:, :], prods[p][1][:, :],
                                op=ADD)
                            accs[p][0] = a1
                    elif m == 3:
                        for p in range(npair):
                            a2 = accpool.tile([128, Wd], F16, name="a2",
                                              tag="acc")
                            nc.vector.tensor_tensor(
                                a2[:, :], prods[p][2][:, :], prods[p][3][:, :],
                                op=ADD)
                            nc.vector.tensor_tensor(
                                accs[p][0][:, :], accs[p][0][:, :], a2[:, :],
                                op=ADD)
                for p in range(npair):
                    a1 = accs[p][0]
                    nc.vector.tensor_tensor(a1[0:64, :], a1[0:64, :],
                                            prods[p][4][:, :], op=ADD)
                    # out-DMA from the idle ACT queue: on the in-order SP
                    # queue it would park behind the next block's window loads
                    t = t0 + 2 * p
                    nc.scalar.dma_start(
                        out=o_dst[:, t * NTILE:t * NTILE + Wd], in_=a1[:, :])
    nc.finalize()
    return nc


_NC = None


def _get_nc():
    global _NC
    if _NC is None:
        _NC = _build()
    return _NC


_RUN_KW = {}  # test harness can inject trace=True etc.
_LAST_RESULT = None


def kernel(gr, gt, Wc, bc):
    global _LAST_RESULT
    gr = np.ascontiguousarray(np.asarray(gr, dtype=np.float32))
    gt = np.ascontiguousarray(np.asarray(gt, dtype=np.float32))
    Wc = np.asarray(Wc, dtype=np.float32)
    bc = np.asarray(bc, dtype=np.float32)

    wb = _prep_w(Wc)
    bb = _prep_b(bc)
    in_maps = [
        {
            "gt_src": _prep_gt(gt[n]),
            "gr_src": _prep_gr(gr[n]),
            "w_src": wb,
            "b_src": bb,
        }
        for n in range(N)
    ]
    res = run_bass_kernel_spmd(
        _get_nc(), in_maps, core_ids=list(range(N)), **_RUN_KW
    )
    _LAST_RESULT = res

    outs = []
    for n in range(N):
        O = res.results[n]["o_dst"].astype(np.float32)
        outs.append((O[:64] + O[64:]).reshape(C, H, W))
    return np.stack(outs).astype(np.float32)
